# revision 3
# baseline (speedup 1.0000x reference)
"""Trainium2 Bass kernel for nn_Correlation (FlowNet-style 1-D correlation).

out[b, d, h, w] = mean_c( left[b,c,h,w] * right[b,c,h,w+d-40] ), d in [0,81),
with right zero-padded along W.  Inputs left/right: [4, 256, 128, 416] fp32.

Strategy (per NeuronCore; the 512 (b,h) rows are sharded over 8 cores by H):
  * out[:, :, h, :] is the 81-wide band of the Gram matrix
    G[w, w'] = sum_c L[c, w] R[c, w'] (contraction C=256 = 2x128 partition
    halves accumulated in fp32 PSUM).  Each 128-column W-tile streams a
    ~208-column window of R through the PE (clipped at the W edges).
  * Inputs are cast to fp16 on the host (halves HBM traffic, 1 PE
    cycle/column); the output is stored fp16 and upcast on the host
    (end-to-end error ~5e-4 absmax-relative, tolerance is 2e-2).
  * Band extraction WITHOUT a DRAM bounce: compute engines cannot apply
    per-partition column shifts, so G's diagonals cannot be gathered
    on-chip -- but a DMA to DRAM can, via a sheared access pattern.
    v5 stores the band DIRECTLY into a padded w-major output layout
    out[h, w, slot], slot in [0, PD), PD = G + 80:
      - rows of each 128-wide W-tile are grouped by G; the PSUM->SBUF copy
        places group a's window (psum cols [G*a, G*a+PD)) at a fixed Bt
        column block, so row p holds out[d, w0+p] at col (p mod G) + d.
      - the store AP [[PD-1, G], [W*PD, nh], [1, WV]] writes row q's whole
        window at slots [(G-1)-q, (G-1)-q+WV): value c maps to
        slot = (G-1) + (c - q) = (G-1) + d.  Off-band cols (slot outside
        [G-1, G+80)) land in the PD-pad of this w or spill into the LOW
        pad of w+1 (row q+1's slots [0, G-1-q) -- never valid there), so
        valid data is written exactly once; the host slices
        slot in [G-1, G+80) and transposes to [B, D, H, W].
    Cost: (G+80)/81 x the band bytes, ~1.4x at G=32 -- vs 3.8x for the
    v3 bounce (sheared write + diagonal reload + transposed store).
  * DMA issues alternate between the SP and ACT HWDGE rings per h-chunk
    so one chunk's sequencer-blocking waits don't stall the next chunk's
    issues (HWDGE waits execute on the issuing sequencer on TRN2).
  * split_dma_waits legalizes Tile's multi-wait instructions for walrus,
    whose NEURON_ISA_TPB_EVENTS descriptor block holds a single sem wait:
    extra waits are hoisted onto the issuing sequencer as one-wait no-ops.
"""

import sys

sys.path.insert(0, "/opt/trn_rl_repo")

from contextlib import ExitStack

import numpy as np

import concourse.bass as bass
import concourse.tile as tile
from concourse import mybir

B, C, H, W = 4, 256, 128, 416
MD = 40
D = 2 * MD + 1  # 81 displacement channels
NCORES = 8
HS = H // NCORES  # 16 H-rows per core

W0S = [0, 128, 256, 384]  # w-tile starts
MS = [128, 128, 128, 32]  # w-tile widths

NH = 4  # h-rows per input DMA / store batch
GROUP = 64  # shear group rows; PD = GROUP + 80 slots per w in the padded out
PD = GROUP + 2 * MD
SLOT0 = GROUP - 1  # host reads slots [SLOT0, SLOT0 + 81)
BUFS = {"inp": 3, "work": 4}


def _windows():
    """Per-tile stream windows over UNPADDED right coords: (a_j, n_j)."""
    res = []
    for w0, m in zip(W0S, MS):
        r0 = w0 - MD
        lo = max(0, -r0)
        hi = min(m + 2 * MD, W - r0)
        res.append((r0 + lo, hi - lo))
    return res


def corr_kernel(tc, outs, ins, hs=HS, nh=NH, grp=GROUP, bufs=None, reps=1,
                rings=None):
    nc = tc.nc
    left, right = ins["left"], ins["right"]
    out = outs["out"]  # [B, hs * W * PD] fp16, w-major padded band

    wins = _windows()
    in_dt = mybir.dt.float16
    psum_n = max(n for _, n in wins)
    bufs = bufs or {}

    # per-tile shear-group geometry
    GJ = [min(grp, m) for m in MS]
    WV = [g + 2 * MD for g in GJ]  # Bt block width per tile
    BT0 = [0]  # Bt col block starts
    for j in range(3):
        BT0.append(BT0[-1] + WV[j])
    BTW = BT0[-1] + WV[3]
    OFF = [W0S[j] - MD - wins[j][0] for j in range(4)]  # psum col of r=w0-40

    rings = rings or {}
    alternate = rings.get("alternate", True)

    def eng(k, parity=0):
        base = {"L": "sync", "R": "sync", "S": "scalar"}
        name = rings.get(k, base[k])
        if alternate and parity % 2 == 1:
            name = {"sync": "scalar", "scalar": "sync"}[name]
        return getattr(nc, name)

    with ExitStack() as ctx:
        inp = ctx.enter_context(tc.tile_pool(name="inp", bufs=bufs.get("inp", 3)))
        work = ctx.enter_context(tc.tile_pool(name="work", bufs=bufs.get("work", 4)))
        psg = ctx.enter_context(
            tc.tile_pool(name="psg", bufs=bufs.get("psg", 6), space="PSUM")
        )

        def one_chunk(b, hc, par):
            L4 = inp.tile([128, 2, nh * W], in_dt, tag="L")
            eng("L", par).dma_start(
                L4[:],
                left[b, :, hc * nh : (hc + 1) * nh, :].rearrange(
                    "(t p) h w -> p t (h w)", p=128
                ),
            )
            R4 = inp.tile([128, 2, nh * W], in_dt, tag="R")
            eng("R", par).dma_start(
                R4[:],
                right[b, :, hc * nh : (hc + 1) * nh, :].rearrange(
                    "(t p) h w -> p t (h w)", p=128
                ),
            )

            Bt = work.tile([128, nh, BTW], in_dt, tag="B")
            ncp = 0
            for hl in range(nh):
                for j in range(4):
                    a, n = wins[j]
                    g = psg.tile([128, psum_n], mybir.dt.float32, tag="g")
                    for t in range(2):
                        hw0 = hl * W
                        lhsT = L4[:, t, hw0 + W0S[j] : hw0 + W0S[j] + MS[j]]
                        rhs = R4[:, t, hw0 + a : hw0 + a + n]
                        nc.tensor.matmul(
                            g[0 : MS[j], 0:n], lhsT, rhs,
                            start=(t == 0), stop=(t == 1),
                        )
                    # scatter psum windows into fixed per-group Bt blocks
                    gj, wv, b0, off = GJ[j], WV[j], BT0[j], OFF[j]
                    for ai in range(MS[j] // gj):
                        r0, r1 = gj * ai, gj * ai + gj
                        base = off + gj * ai  # psum col of Bt col 0
                        c_lo = max(0, -base)
                        c_hi = min(wv, n - base)
                        if c_lo > 0:
                            nc.vector.memset(
                                Bt[r0:r1, hl, b0 : b0 + c_lo], 0.0
                            )
                        if c_hi < wv:
                            nc.vector.memset(
                                Bt[r0:r1, hl, b0 + c_hi : b0 + wv], 0.0
                            )
                        src = g[r0:r1, base + c_lo : base + c_hi]
                        dst = Bt[r0:r1, hl, b0 + c_lo : b0 + c_hi]
                        if ncp % 2 == 0:  # split copies across DVE / ACT
                            nc.vector.tensor_scalar_mul(dst, src, 1.0 / C)
                        else:
                            nc.scalar.mul(dst, src, 1.0 / C)
                        ncp += 1

            # direct sheared store into the padded out: row (a, q) writes
            # w = w0 + G*a + q, slots [(G-1)-q, (G-1)-q+WV).  Spill past
            # slot PD-1 lands in w+1's low pad (never valid); the last row
            # of each group (q = G-1 resp. the j3 edge) does not spill, so
            # spills stay inside one DMA.
            ch0 = hc * nh * W * PD
            for j in range(4):
                gj, wv, b0 = GJ[j], WV[j], BT0[j]
                for ai in range(MS[j] // gj):
                    w0 = W0S[j] + gj * ai
                    dst = out[b, ch0 + w0 * PD + SLOT0 - 0 :]
                    dd = dst.ap
                    dd.clear()
                    dd.extend([[PD - 1, gj], [W * PD, nh], [1, wv]])
                    dst.ap = dd
                    eng("S", par).dma_start(
                        dst, Bt[gj * ai : gj * ai + gj, :, b0 : b0 + wv]
                    )

        assert hs % nh == 0
        ci = 0
        for _rep in range(reps):
            for b in range(B):
                for hc in range(hs // nh):
                    one_chunk(b, hc, ci)
                    ci += 1


def split_dma_waits(nc):
    """Legalize for walrus: instruction descriptors hold ONE sync wait
    (NEURON_ISA_TPB_EVENTS), but Tile attaches up to ~3.  Move the extras to
    standalone InstEventSemaphore waits on the instruction's engine right
    before it -- sequencers execute (and enqueue HWDGE descriptors) in
    program order, so the hoisted waits still guard the instruction."""
    n = 0
    for fn in nc.m.functions:
        for bb in fn.blocks:
            insts = bb.instructions
            out = []
            for inst in insts:
                si = getattr(inst, "sync_info", None)
                eng = getattr(inst, "engine", None)
                if (
                    si is not None
                    and si.on_wait
                    and len(si.on_wait) > 1
                    and eng is not None
                    and eng != mybir.EngineType.Unassigned
                ):
                    waits = list(si.on_wait)
                    for w in waits[:-1]:
                        ev = mybir.InstNoOp(name=f"{inst.name}-prewait{n}")
                        ev.engine = eng
                        ev.sync_info = mybir.SyncInfo(on_wait=[w], on_update=[])
                        nc.register_instruction(ev)
                        out.append(ev)
                        n += 1
                    inst.sync_info = mybir.SyncInfo(
                        on_wait=waits[-1:], on_update=list(si.on_update or [])
                    )
                out.append(inst)
            bb.instructions = out
    return n


def build_nc(hs=HS, nh=NH, grp=GROUP, reps=1, bufs=None):
    in_dt = mybir.dt.float16
    nc = bass.Bass(
        trn_type="TRN2", target_bir_lowering=False, debug=False, num_devices=NCORES
    )
    pd = grp + 2 * MD
    ins = {
        "left": nc.dram_tensor("left", [B, C, hs, W], in_dt, kind="ExternalInput").ap(),
        "right": nc.dram_tensor(
            "right", [B, C, hs, W], in_dt, kind="ExternalInput"
        ).ap(),
    }
    outs = {
        "out": nc.dram_tensor(
            "out", [B, hs * W * pd], in_dt, kind="ExternalOutput"
        ).ap()
    }
    with tile.TileContext(nc) as tc:
        corr_kernel(
            tc, outs, ins, hs=hs, nh=nh, grp=grp, bufs=(bufs or BUFS), reps=reps
        )
    split_dma_waits(nc)
    return nc


def make_in_maps(left, right):
    in_maps = []
    for i in range(NCORES):
        sl = slice(i * HS, (i + 1) * HS)
        in_maps.append(
            {
                "left": np.ascontiguousarray(left[:, :, sl, :]).astype(np.float16),
                "right": np.ascontiguousarray(right[:, :, sl, :]).astype(np.float16),
            }
        )
    return in_maps


def extract_out(flat, grp=GROUP):
    """[B, hs*W*PD] fp16 padded band -> [B, D, hs, W] fp32."""
    pd = grp + 2 * MD
    s0 = grp - 1
    a = np.asarray(flat).reshape(B, HS, W, pd)[:, :, :, s0 : s0 + D]
    return np.ascontiguousarray(a.transpose(0, 3, 1, 2)).astype(np.float32)


def kernel(left, right):
    """Full-input entry point: [4,256,128,416] fp32 x2 -> [4,81,128,416] fp32."""
    from concourse.bass_utils import run_bass_kernel_spmd

    left = np.asarray(left, dtype=np.float32)
    right = np.asarray(right, dtype=np.float32)
    nc = build_nc()
    in_maps = make_in_maps(left, right)
    res = run_bass_kernel_spmd(nc, in_maps, list(range(NCORES)))
    return np.concatenate(
        [extract_out(res.results[i]["out"]) for i in range(NCORES)], axis=2
    )


if __name__ == "__main__":
    rng = np.random.default_rng(0)
    lf = rng.standard_normal((B, C, H, W), dtype=np.float32)
    rt = rng.standard_normal((B, C, H, W), dtype=np.float32)
    o = kernel(left=lf, right=rt)
    print(o.shape, o.dtype)
